# revision 1
# baseline (speedup 1.0000x reference)
"""Causal single-head attention on 8 trn2 NeuronCores.

Problem: x[4, 2048, 1024] fp32, W_q/W_k/W_v [1024, 1024] fp32 (torch Linear
layout, y = x @ W.T). Causal softmax attention, d_out = 1024.

Sharding: data-parallel over batch (4) x 2-way KEY split per batch.
Core c = 2*b + h handles batch b and the eight 128-row KEY blocks
{h, h+2, ..., h+14} (global 128-row block indices). Each core computes
Q for ALL 2048 queries but K/V only for its 1024 keys, runs flash-style
attention of all queries against its keys (exp without max subtraction —
scores are bounded — so partial sums merge exactly), and outputs the
unnormalized partial AV (bf16) and partial denominator (fp32). The host
merges: out = (AV_0 + AV_1) / (den_0 + den_1). Duplicating only Q (one
tensor) instead of K and V (two) minimizes the no-communication
projection cost, and interleaving key blocks at 128-row granularity makes
the local causal extent for query superblock m exactly 128*(m+1) on BOTH
cores of a pair — one SPMD program, zero static extent waste at tile
granularity, and only the last local key tile of each superblock needs a
causal mask, which is a single m-independent [128, 256] additive tile
whose data depends only on h (supplied as input).

Per-core device program (matmul operands bf16, fp32 PSUM accumulate):
  1. Projections: V[skl, o], K^T[o, skl] from gathered-key x^T; Q^T[o, sq]
     from full x^T. A short dependency-free matmul warmup keeps the PE
     clock un-gated while the first DMAs land.
  2. Attention, key-stationary scores pass: per local key tile t, scores
     S^T[128, w] against all query superblocks m >= t in 512-wide matmul
     chunks, diagonal mask add, exp (ACT, scale 1/32) into resident bf16
     es tiles. Query-stationary AV pass: per superblock m, accumulate AV
     and the softmax denominator (PE ones-matmul) in PSUM over tiles
     0..m, evacuate av as bf16 + den as fp32.
"""

import copy

import numpy as np
import ml_dtypes

import concourse.bass as bass
import concourse.mybir as mybir
import concourse.tile as tile
from concourse.bass_utils import run_bass_kernel_spmd

BF16 = mybir.dt.bfloat16
F32 = mybir.dt.float32

B, S, D = 4, 2048, 1024
N_CORES = 8
SB = 256            # query superblock rows / key gather block
N_SB = S // SB      # 8 query superblocks per core
SKL = S // 2        # local keys per core (1024)
MASK_NEG = -1.0e5


def _legalize_waits(nc):
    """Split multi-wait instructions into single-wait NOP chains.

    The walrus here accepts at most one sync-wait command per instruction,
    while TileContext emits several `on_wait` entries on one instruction.
    Hoist all but the last wait onto same-engine NOPs placed immediately
    before the instruction; the engine sequencer stalls on each in order.
    """
    uid = 0
    for fn in nc.m.functions:
        for bb in fn.blocks:
            out = []
            for inst in bb.instructions:
                si = inst.sync_info
                waits = list(si.on_wait) if si and si.on_wait else []
                if len(waits) > 1:
                    for w in waits[:-1]:
                        nop = mybir.InstNoOp(name=f"waitsplit_{uid}", ins=[], outs=[])
                        uid += 1
                        nop.engine = inst.engine
                        si2 = copy.deepcopy(si)
                        si2.on_wait = [w]
                        si2.on_update = []
                        nop.sync_info = si2
                        out.append(nop)
                    si.on_wait = waits[-1:]
                    inst.sync_info = si
                out.append(inst)
            bb.instructions = out


def build_nc(reps=1):
    nc = bass.Bass("TRN2", target_bir_lowering=False, debug=False, num_devices=N_CORES)

    xT_d = nc.dram_tensor("xT", [D, S], BF16, kind="ExternalInput")      # all queries
    xkT_d = nc.dram_tensor("xkT", [D, SKL], BF16, kind="ExternalInput")  # gathered keys
    wqT_d = nc.dram_tensor("wqT", [D, D], BF16, kind="ExternalInput")
    wkT_d = nc.dram_tensor("wkT", [D, D], BF16, kind="ExternalInput")
    wvT_d = nc.dram_tensor("wvT", [D, D], BF16, kind="ExternalInput")
    # additive causal mask for the last 128 local keys of a superblock,
    # [128 keys, 256 queries]; data depends only on h
    mask_d = nc.dram_tensor("maskT", [128, SB], F32, kind="ExternalInput")
    yav_d = nc.dram_tensor("yav", [S, D], BF16, kind="ExternalOutput")
    yden_d = nc.dram_tensor("yden", [S, 1], F32, kind="ExternalOutput")

    ND = D // 128       # 8 d-tiles
    NO = D // 128       # 8 o-tiles
    NSKL = SKL // 128   # 8 local key tiles

    with tile.TileContext(nc) as tc:
        with (
            tc.tile_pool(name="xT", bufs=ND) as xT_pool,
            tc.tile_pool(name="xkT", bufs=ND) as xkT_pool,
            tc.tile_pool(name="w", bufs=2 * ND) as w_pool,
            tc.tile_pool(name="KT", bufs=NO) as KT_pool,
            tc.tile_pool(name="V", bufs=NSKL) as V_pool,
            tc.tile_pool(name="QT", bufs=NO) as QT_pool,
            tc.tile_pool(name="mask", bufs=1) as mask_pool,
            tc.tile_pool(name="ones", bufs=1) as ones_pool,
            tc.tile_pool(name="es", bufs=3) as es_pool,
            tc.tile_pool(name="oav", bufs=6) as oav_pool,
            tc.tile_pool(name="oden", bufs=4) as oden_pool,
            tc.tile_pool(name="warm", bufs=2) as warm_pool,
        ):
            for rep in range(reps):
                # ---- HAM warmup: dependency-free matmuls keep PE busy during
                # the initial DMA wait so phase 1 starts at full clock
                wsrc = warm_pool.tile([128, 512], BF16, tag="wsrc", name=f"wsrc{rep}")
                nc.gpsimd.memset(wsrc[:], 0.0)
                with tc.tile_pool(name="wps", bufs=1, space="PSUM") as wps_pool:
                    wps = wps_pool.tile([128, 512], F32, tag="wps", name=f"wps{rep}")
                    for i in range(19):
                        nc.tensor.matmul(
                            wps[:], wsrc[:, 0:128], wsrc[:], start=(i == 0),
                            stop=(i == 18),
                        )
                # ---- loads, in consumption order: V <- (wv, xkT); KT <- wk; QT <- (xT, wq)
                wv_t = []
                for i in range(ND):
                    t = w_pool.tile([128, D], BF16, tag="w", name=f"wv{i}")
                    nc.sync.dma_start(t[:], wvT_d[i * 128:(i + 1) * 128, :])
                    wv_t.append(t)
                xk_t = []
                for i in range(ND):
                    t = xkT_pool.tile([128, SKL], BF16, tag="xkT", name=f"xk{i}")
                    nc.sync.dma_start(t[:], xkT_d[i * 128:(i + 1) * 128, :])
                    xk_t.append(t)
                wk_t = []
                for i in range(ND):
                    t = w_pool.tile([128, D], BF16, tag="w", name=f"wk{i}")
                    nc.sync.dma_start(t[:], wkT_d[i * 128:(i + 1) * 128, :])
                    wk_t.append(t)
                xT_t = []
                for i in range(ND):
                    t = xT_pool.tile([128, S], BF16, tag="xT", name=f"xq{i}")
                    nc.sync.dma_start(t[:], xT_d[i * 128:(i + 1) * 128, :])
                    xT_t.append(t)
                wq_t = []
                for i in range(ND):
                    t = w_pool.tile([128, D], BF16, tag="w", name=f"wq{i}")
                    nc.sync.dma_start(t[:], wqT_d[i * 128:(i + 1) * 128, :])
                    wq_t.append(t)
                mask_t = []
                for i in range(1):
                    t = mask_pool.tile([128, SB], F32, tag="mask", name=f"mask{i}")
                    nc.sync.dma_start(t[:], mask_d[i * 128:(i + 1) * 128, :])
                    mask_t.append(t)
                ones_t = ones_pool.tile([128, 1], BF16)
                nc.gpsimd.memset(ones_t[:], 1.0)

                # ---- phase 1: projections ----
                with tc.tile_pool(name="psum1", bufs=4, space="PSUM") as psum1:
                    # V[skl, o] = sum_d xkT[d, skl] * wvT[d, o]
                    V_t = [
                        V_pool.tile([128, D], BF16, tag="V", name=f"V{st}") for st in range(NSKL)
                    ]
                    for st in range(NSKL):
                        for oc in range(D // 512):
                            ps = psum1.tile([128, 512], F32, tag="ps1", name=f"psv{st}_{oc}")
                            for d in range(ND):
                                nc.tensor.matmul(
                                    ps[:],
                                    xk_t[d][:, st * 128:(st + 1) * 128],
                                    wv_t[d][:, oc * 512:(oc + 1) * 512],
                                    start=(d == 0),
                                    stop=(d == ND - 1),
                                )
                            nc.vector.tensor_copy(V_t[st][:, oc * 512:(oc + 1) * 512], ps[:])
                    # K^T[o, skl] = sum_d wkT[d, o] * xkT[d, skl]
                    KT_t = [
                        KT_pool.tile([128, SKL], BF16, tag="KT", name=f"KT{ot}") for ot in range(NO)
                    ]
                    for skc in range(SKL // 512):
                        for ot in range(NO):
                            ps = psum1.tile([128, 512], F32, tag="ps1", name=f"psk{skc}_{ot}")
                            for d in range(ND):
                                nc.tensor.matmul(
                                    ps[:],
                                    wk_t[d][:, ot * 128:(ot + 1) * 128],
                                    xk_t[d][:, skc * 512:(skc + 1) * 512],
                                    start=(d == 0),
                                    stop=(d == ND - 1),
                                )
                            nc.vector.tensor_copy(
                                KT_t[ot][:, skc * 512:(skc + 1) * 512], ps[:]
                            )
                    # Q^T[o, sq] = sum_d wqT[d, o] * xT[d, sq]   (all 2048 queries)
                    QT_t = [
                        QT_pool.tile([128, S], BF16, tag="QT", name=f"QT{ot}") for ot in range(NO)
                    ]
                    for sqc in range(S // 512):
                        for ot in range(NO):
                            ps = psum1.tile([128, 512], F32, tag="ps1", name=f"psq{sqc}_{ot}")
                            for d in range(ND):
                                nc.tensor.matmul(
                                    ps[:],
                                    wq_t[d][:, ot * 128:(ot + 1) * 128],
                                    xT_t[d][:, sqc * 512:(sqc + 1) * 512],
                                    start=(d == 0),
                                    stop=(d == ND - 1),
                                )
                            nc.vector.tensor_copy(
                                QT_t[ot][:, sqc * 512:(sqc + 1) * 512], ps[:]
                            )

                # ---- phase 2: attention (partial over local keys) ----
                # Pass A (key-stationary): for each local key tile t, scores
                # against ALL query superblocks m >= t in 512-wide chunks,
                # mask + exp into resident es tiles. Pass B (query-stationary):
                # per superblock, accumulate AV and denominator in PSUM.
                with (
                    tc.tile_pool(name="av", bufs=4, space="PSUM") as av_pool,
                    tc.tile_pool(name="pss", bufs=2, space="PSUM") as pss_pool,
                    tc.tile_pool(name="den", bufs=2, space="PSUM") as den_pool,
                ):
                    es_t = []
                    for t in range(NSKL):
                        w = S - SB * t          # queries [SB*t, S)
                        es = es_pool.tile([128, w], BF16, tag=f"es{t}", bufs=1,
                                          name=f"es{rep}_{t}")
                        es_t.append(es)
                        for c0 in range(0, w, 512):
                            cw = min(512, w - c0)
                            ps_s = pss_pool.tile([128, 512], F32, tag="pss",
                                                 name=f"pss{t}_{c0}")
                            for od in range(NO):
                                nc.tensor.matmul(
                                    ps_s[:, 0:cw],
                                    KT_t[od][:, t * 128:(t + 1) * 128],
                                    QT_t[od][:, SB * t + c0:SB * t + c0 + cw],
                                    start=(od == 0),
                                    stop=(od == NO - 1),
                                )
                            if c0 == 0:
                                # first 256 queries belong to superblock m=t:
                                # causal boundary mask for the diagonal tile
                                nc.vector.tensor_add(
                                    ps_s[:, 0:SB], ps_s[:, 0:SB], mask_t[0][:]
                                )
                            nc.scalar.activation(
                                es[:, c0:c0 + cw], ps_s[:, 0:cw],
                                mybir.ActivationFunctionType.Exp,
                                scale=1.0 / 32.0,
                            )
                    for m in range(N_SB):
                        n = m + 1              # local sk-tiles this superblock
                        avs = [
                            av_pool.tile([128, 512], F32, tag="av", name=f"av{m}_{i}")
                            for i in range(4)
                        ]
                        dens = [
                            den_pool.tile([128, 1], F32, tag="den", name=f"den{m}_{i}")
                            for i in range(2)
                        ]
                        # tile-major: finish each av tile's accumulation
                        # before starting the next, so its evacuation overlaps
                        # the remaining AV matmuls instead of serializing after
                        # the superblock's last matmul
                        for sqh in range(2):
                            for t in range(n):
                                q0 = SB * (m - t)
                                nc.tensor.matmul(
                                    dens[sqh][:],
                                    es_t[t][:, q0 + sqh * 128:q0 + (sqh + 1) * 128],
                                    ones_t[:],
                                    start=(t == 0),
                                    stop=(t == n - 1),
                                )
                        for sqh in range(2):
                            for oh in range(2):
                                for t in range(n):
                                    q0 = SB * (m - t)
                                    nc.tensor.matmul(
                                        avs[sqh * 2 + oh][:],
                                        es_t[t][:, q0 + sqh * 128:q0 + (sqh + 1) * 128],
                                        V_t[t][:, oh * 512:(oh + 1) * 512],
                                        start=(t == 0),
                                        stop=(t == n - 1),
                                    )
                        for sqh in range(2):
                            r0 = m * SB + sqh * 128
                            oden = oden_pool.tile([128, 1], F32, tag="oden", name=f"oden{m}_{sqh}")
                            nc.vector.tensor_copy(oden[:], dens[sqh][:])
                            nc.sync.dma_start(yden_d[r0:r0 + 128, :], oden[:])
                            oav = oav_pool.tile([128, D], BF16, tag="oav", name=f"oav{m}_{sqh}")
                            for oh in range(2):
                                nc.vector.tensor_copy(
                                    oav[:, oh * 512:(oh + 1) * 512], avs[sqh * 2 + oh][:]
                                )
                                if m == N_SB - 1:
                                    # kernel tail: ship each half as soon as
                                    # its copy lands
                                    nc.sync.dma_start(
                                        yav_d[r0:r0 + 128, oh * 512:(oh + 1) * 512],
                                        oav[:, oh * 512:(oh + 1) * 512],
                                    )
                            if m != N_SB - 1:
                                nc.sync.dma_start(yav_d[r0:r0 + 128, :], oav[:])

    _legalize_waits(nc)
    return nc


_NC_CACHE = None


def _get_nc():
    global _NC_CACHE
    if _NC_CACHE is None:
        _NC_CACHE = build_nc()
    return _NC_CACHE


def _prep_core_inputs(x, wqT, wkT, wvT, b, h):
    xb = np.ascontiguousarray(x[b])                       # [S, D] fp32
    xT = np.ascontiguousarray(xb.T).astype(ml_dtypes.bfloat16)
    # keys interleave at 128-row granularity: core h owns global 128-blocks
    # {h, h+2, ..., h+14}; superblock m's local extent is exactly 128*(m+1)
    # for both cores, and only the last local key tile needs masking.
    blocks = [h + 2 * i for i in range(8)]
    xk = np.concatenate([xb[128 * t:128 * (t + 1)] for t in blocks], axis=0)
    xkT = np.ascontiguousarray(xk.T).astype(ml_dtypes.bfloat16)
    # mask for the last local key tile (global block 2m+h vs queries of
    # superblock m): keep iff 128*h + r_k <= r_q
    kk = np.arange(128)[:, None]
    qq = np.arange(SB)[None, :]
    maskT = np.where(128 * h + kk <= qq, 0.0, MASK_NEG).astype(np.float32)
    return {
        "xT": xT, "xkT": xkT,
        "wqT": wqT, "wkT": wkT, "wvT": wvT,
        "maskT": maskT,
    }


def kernel(x, W_q, W_k, W_v):
    x = np.asarray(x, dtype=np.float32)
    wqT = np.ascontiguousarray(np.asarray(W_q, np.float32).T).astype(ml_dtypes.bfloat16)
    wkT = np.ascontiguousarray(np.asarray(W_k, np.float32).T).astype(ml_dtypes.bfloat16)
    wvT = np.ascontiguousarray(np.asarray(W_v, np.float32).T).astype(ml_dtypes.bfloat16)

    in_maps = []
    for c in range(N_CORES):
        b, h = divmod(c, 2)
        in_maps.append(_prep_core_inputs(x, wqT, wkT, wvT, b, h))

    nc = _get_nc()
    res = run_bass_kernel_spmd(nc, in_maps, list(range(N_CORES)))

    out = np.empty((B, S, D), dtype=np.float32)
    for b in range(B):
        av0 = np.asarray(res.results[2 * b]["yav"], dtype=np.float32)
        av1 = np.asarray(res.results[2 * b + 1]["yav"], dtype=np.float32)
        den = res.results[2 * b]["yden"] + res.results[2 * b + 1]["yden"]
        out[b] = (av0 + av1) / den
    return out



# revision 6
# speedup vs baseline: 1.1955x; 1.1955x over previous
"""Causal single-head attention on 8 trn2 NeuronCores.

Problem: x[4, 2048, 1024] fp32, W_q/W_k/W_v [1024, 1024] fp32 (torch Linear
layout, y = x @ W.T). Causal softmax attention, d_out = 1024.

Sharding: data-parallel over batch (4) x 2-way KEY split per batch.
Core c = 2*b + h handles batch b and the eight 128-row KEY blocks
{h, h+2, ..., h+14} (global 128-row block indices). Each core needs Q for
ALL 2048 queries but K/V only for its 1024 keys, runs flash-style
attention of all queries against its keys (exp without max subtraction —
scores are bounded — so partial sums merge exactly), and outputs the
unnormalized partial AV (bf16) and partial denominator (fp32). The host
merges: out = (AV_0 + AV_1) / (den_0 + den_1). Interleaving key blocks at
128-row granularity makes the local causal extent for query superblock m
exactly 128*(m+1) on BOTH cores of a pair — one SPMD program, zero static
extent waste at tile granularity, and only the last local key tile of each
superblock needs a causal mask, a single m-independent [128, 256] additive
tile whose data depends only on h (supplied as input).

Q-projection dedup: instead of both cores of a pair computing the full
Q^T (8 o-tiles), each core computes only 4 o-tiles — which ones is data:
core h receives wqhT = W_q^T[:, 512h:512(h+1)] — writes them to a DRAM
bounce buffer, and an in-pair AllGather assembles the full Q^T[1024, 2048]
(rank order puts o-dims 0..511 from core 2b first, so the gathered block
order is the natural o order on both cores). The gather + reload overlap
the V and K projections. This removes 65536 of ~410k PE cycles per core.

Per-core device program (matmul operands bf16, fp32 PSUM accumulate):
  1. Warmup: dependency-free matmuls keep the PE clock un-gated while the
     first DMAs land. Q-own: Q^T_own[512, 2048] from x^T (loaded in 512-
     column chunks so the first psum group completes after ~2 MB of DMA),
     evacuated to DRAM, AllGather within the pair, full Q^T DMA'd back.
     Meanwhile: V[skl, o], K^T[o, skl] from gathered-key x^T.
  2. Attention, key-stationary scores pass: per local key tile t, scores
     S^T[128, w] against all query superblocks m >= t in 512-wide matmul
     chunks, diagonal mask add, exp (ACT, scale 1/32) into resident bf16
     es tiles. Query-stationary AV pass: per superblock m, accumulate AV
     and the softmax denominator (PE ones-matmul) in PSUM over tiles
     0..m, evacuate av as bf16 + den as fp32.
"""

import copy
from types import SimpleNamespace

import numpy as np
import ml_dtypes

import concourse.bass as bass
import concourse.mybir as mybir
import concourse.tile as tile
from concourse.bass_utils import run_bass_kernel_spmd

BF16 = mybir.dt.bfloat16
F32 = mybir.dt.float32

B, S, D = 4, 2048, 1024
N_CORES = 8
SB = 256            # query superblock rows / key gather block
N_SB = S // SB      # 8 query superblocks per core
SKL = S // 2        # local keys per core (1024)
MASK_NEG = -1.0e5

ND = D // 128       # 8 d-tiles
NO = D // 128       # 8 o-tiles
NSKL = SKL // 128   # 8 local key tiles
NQC = S // 512      # 4 query column chunks


def _legalize_waits(nc):
    """Split multi-wait instructions into single-wait NOP chains.

    The walrus here accepts at most one sync-wait command per instruction,
    while TileContext emits several `on_wait` entries on one instruction.
    Hoist all but the last wait onto same-engine NOPs placed immediately
    before the instruction; the engine sequencer stalls on each in order.
    """
    uid = 0
    for fn in nc.m.functions:
        for bb in fn.blocks:
            out = []
            for inst in bb.instructions:
                si = inst.sync_info
                waits = list(si.on_wait) if si and si.on_wait else []
                if len(waits) > 1:
                    for w in waits[:-1]:
                        nop = mybir.InstNoOp(name=f"waitsplit_{uid}", ins=[], outs=[])
                        uid += 1
                        nop.engine = inst.engine
                        si2 = copy.deepcopy(si)
                        si2.on_wait = [w]
                        si2.on_update = []
                        nop.sync_info = si2
                        out.append(nop)
                    si.on_wait = waits[-1:]
                    inst.sync_info = si
                out.append(inst)
            bb.instructions = out


def _emit_warmup(e, rep):
    # dependency-free matmuls keep PE busy during the initial DMA wait so
    # phase 1 starts at full clock
    nc, tc = e.nc, e.tc
    wsrc = e.warm_pool.tile([128, 512], BF16, tag="wsrc", name=f"wsrc{rep}")
    nc.gpsimd.memset(wsrc[:], 0.0)
    with tc.tile_pool(name="wps", bufs=1, space="PSUM") as wps_pool:
        wps = wps_pool.tile([128, 512], F32, tag="wps", name=f"wps{rep}")
        for i in range(19):
            nc.tensor.matmul(
                wps[:], wsrc[:, 0:128], wsrc[:], start=(i == 0), stop=(i == 18)
            )


def _emit_loads(e, rep):
    # loads, in consumption order:
    # Q-own <- (wqh, xT chunks); V <- (wv, xk); K <- wk
    nc = e.nc
    e.wqh_t = []
    for i in range(ND):
        t = e.wqh_pool.tile([128, D // 2], BF16, tag="wqh", name=f"wqh{i}")
        nc.sync.dma_start(t[:], e.wqhT_d[i * 128:(i + 1) * 128, :])
        e.wqh_t.append(t)
    # xT in 512-column chunks, sqc-major so Q-own streams tile-by-tile
    e.xT_t = [[None] * ND for _ in range(NQC)]
    for sqc in range(NQC):
        for i in range(ND):
            t = e.xT_pool.tile([128, 512], BF16, tag="xT", name=f"xq{sqc}_{i}")
            nc.sync.dma_start(
                t[:], e.xT_d[i * 128:(i + 1) * 128, sqc * 512:(sqc + 1) * 512]
            )
            e.xT_t[sqc][i] = t
    e.wv_t = []
    for i in range(ND):
        t = e.w_pool.tile([128, D], BF16, tag="w", name=f"wv{i}")
        nc.sync.dma_start(t[:], e.wvT_d[i * 128:(i + 1) * 128, :])
        e.wv_t.append(t)
    e.xk_t = []
    for i in range(ND):
        t = e.xkT_pool.tile([128, SKL], BF16, tag="xkT", name=f"xk{i}")
        nc.sync.dma_start(t[:], e.xkT_d[i * 128:(i + 1) * 128, :])
        e.xk_t.append(t)
    e.wk_t = []
    for i in range(ND):
        t = e.w_pool.tile([128, D], BF16, tag="w", name=f"wk{i}")
        nc.sync.dma_start(t[:], e.wkT_d[i * 128:(i + 1) * 128, :])
        e.wk_t.append(t)
    e.mask_t = e.mask_pool.tile([128, SB], F32, tag="mask", name="mask0")
    nc.sync.dma_start(e.mask_t[:], e.mask_d[0:128, :])
    e.ones_t = e.ones_pool.tile([128, 1], BF16)
    nc.gpsimd.memset(e.ones_t[:], 1.0)


def _emit_q_own(e, rep, psum1):
    # Q-own^T[o_own, sq]: 4 o-tiles of own half, all queries -> DRAM bounce
    nc = e.nc
    qin_d = e.qdram_pool.tile([D // 2, S], BF16, tag=f"qin{rep}", bufs=1,
                              name=f"qin{rep}")
    qout_d = e.qdram_pool.tile([D, S], BF16, tag=f"qout{rep}", bufs=1,
                               name=f"qout{rep}")
    for sqc in range(NQC):
        for ot in range(NO // 2):
            ps = psum1.tile([128, 512], F32, tag="ps1", name=f"psq{sqc}_{ot}")
            for d in range(ND):
                nc.tensor.matmul(
                    ps[:],
                    e.wqh_t[d][:, ot * 128:(ot + 1) * 128],
                    e.xT_t[sqc][d][:],
                    start=(d == 0),
                    stop=(d == ND - 1),
                )
            qe = e.qe_pool.tile([128, 512], BF16, tag="qe", name=f"qe{sqc}_{ot}")
            nc.vector.tensor_copy(qe[:], ps[:])
            nc.sync.dma_start(
                qin_d[ot * 128:(ot + 1) * 128, sqc * 512:(sqc + 1) * 512], qe[:]
            )
    # in-pair allgather: rank 2b's half (o 0..511) lands first, so gathered
    # block j == o-tile j on both cores
    if e.sim_no_cc:
        nc.sync.dma_start(qout_d[0:D // 2, :], qin_d[:])
        nc.sync.dma_start(qout_d[D // 2:D, :], qin_d[:])
    else:
        nc.gpsimd.collective_compute(
            "AllGather",
            mybir.AluOpType.bypass,
            replica_groups=[[0, 1], [2, 3], [4, 5], [6, 7]],
            ins=[qin_d.opt()],
            outs=[qout_d.opt()],
        )
    e.QT_t = [
        e.QT_pool.tile([128, S], BF16, tag="QT", name=f"QT{ot}")
        for ot in range(NO)
    ]
    for ot in range(NO):
        nc.sync.dma_start(e.QT_t[ot][:], qout_d[ot * 128:(ot + 1) * 128, :])


def _emit_kv(e, psum1):
    nc = e.nc
    # V[skl, o] = sum_d xkT[d, skl] * wvT[d, o]
    e.V_t = [e.V_pool.tile([128, D], BF16, tag="V", name=f"V{st}")
             for st in range(NSKL)]
    for st in range(NSKL):
        for oc in range(D // 512):
            ps = psum1.tile([128, 512], F32, tag="ps1", name=f"psv{st}_{oc}")
            for d in range(ND):
                nc.tensor.matmul(
                    ps[:],
                    e.xk_t[d][:, st * 128:(st + 1) * 128],
                    e.wv_t[d][:, oc * 512:(oc + 1) * 512],
                    start=(d == 0),
                    stop=(d == ND - 1),
                )
            nc.vector.tensor_copy(e.V_t[st][:, oc * 512:(oc + 1) * 512], ps[:])
    # K^T[o, skl] = sum_d wkT[d, o] * xkT[d, skl]
    e.KT_t = [e.KT_pool.tile([128, SKL], BF16, tag="KT", name=f"KT{ot}")
              for ot in range(NO)]
    for skc in range(SKL // 512):
        for ot in range(NO):
            ps = psum1.tile([128, 512], F32, tag="ps1", name=f"psk{skc}_{ot}")
            for d in range(ND):
                nc.tensor.matmul(
                    ps[:],
                    e.wk_t[d][:, ot * 128:(ot + 1) * 128],
                    e.xk_t[d][:, skc * 512:(skc + 1) * 512],
                    start=(d == 0),
                    stop=(d == ND - 1),
                )
            nc.vector.tensor_copy(e.KT_t[ot][:, skc * 512:(skc + 1) * 512], ps[:])


def _emit_scores(e, rep, pss_pool):
    # key-stationary scores pass: per local key tile t, scores against all
    # query superblocks m >= t in 512-wide chunks, mask + exp into resident
    # bf16 es tiles
    nc = e.nc
    e.es_t = []
    for t in range(NSKL):
        w = S - SB * t          # queries [SB*t, S)
        es = e.es_pool.tile([128, w], BF16, tag=f"es{t}", bufs=1,
                            name=f"es{rep}_{t}")
        e.es_t.append(es)
        for c0 in range(0, w, 512):
            cw = min(512, w - c0)
            ps_s = pss_pool.tile([128, 512], F32, tag="pss", name=f"pss{t}_{c0}")
            for od in range(NO):
                nc.tensor.matmul(
                    ps_s[:, 0:cw],
                    e.KT_t[od][:, t * 128:(t + 1) * 128],
                    e.QT_t[od][:, SB * t + c0:SB * t + c0 + cw],
                    start=(od == 0),
                    stop=(od == NO - 1),
                )
            if c0 == 0:
                # first 256 queries belong to superblock m=t: causal
                # boundary mask for the diagonal tile
                nc.vector.tensor_add(ps_s[:, 0:SB], ps_s[:, 0:SB], e.mask_t[:])
            nc.scalar.activation(
                es[:, c0:c0 + cw], ps_s[:, 0:cw],
                mybir.ActivationFunctionType.Exp,
                scale=1.0 / 32.0,
            )


def _emit_av(e, av_pool, den_pool):
    # query-stationary AV pass: per superblock m, accumulate AV and the
    # softmax denominator (PE ones-matmul) in PSUM over tiles 0..m
    nc = e.nc
    for m in range(N_SB):
        n = m + 1              # local sk-tiles this superblock
        avs = [av_pool.tile([128, 512], F32, tag="av", name=f"av{m}_{i}")
               for i in range(4)]
        dens = [den_pool.tile([128, 1], F32, tag="den", name=f"den{m}_{i}")
                for i in range(2)]
        # tile-major: finish each av tile's accumulation before starting the
        # next, so its evacuation overlaps the remaining AV matmuls instead
        # of serializing after the superblock's last matmul
        for sqh in range(2):
            for t in range(n):
                q0 = SB * (m - t)
                nc.tensor.matmul(
                    dens[sqh][:],
                    e.es_t[t][:, q0 + sqh * 128:q0 + (sqh + 1) * 128],
                    e.ones_t[:],
                    start=(t == 0),
                    stop=(t == n - 1),
                )
        for sqh in range(2):
            for oh in range(2):
                for t in range(n):
                    q0 = SB * (m - t)
                    nc.tensor.matmul(
                        avs[sqh * 2 + oh][:],
                        e.es_t[t][:, q0 + sqh * 128:q0 + (sqh + 1) * 128],
                        e.V_t[t][:, oh * 512:(oh + 1) * 512],
                        start=(t == 0),
                        stop=(t == n - 1),
                    )
        for sqh in range(2):
            r0 = m * SB + sqh * 128
            oden = e.oden_pool.tile([128, 1], F32, tag="oden",
                                    name=f"oden{m}_{sqh}")
            nc.vector.tensor_copy(oden[:], dens[sqh][:])
            nc.sync.dma_start(e.yden_d[r0:r0 + 128, :], oden[:])
            oav = e.oav_pool.tile([128, D], BF16, tag="oav", name=f"oav{m}_{sqh}")
            for oh in range(2):
                nc.vector.tensor_copy(
                    oav[:, oh * 512:(oh + 1) * 512], avs[sqh * 2 + oh][:]
                )
                if m == N_SB - 1:
                    # kernel tail: ship each half as soon as its copy lands
                    nc.sync.dma_start(
                        e.yav_d[r0:r0 + 128, oh * 512:(oh + 1) * 512],
                        oav[:, oh * 512:(oh + 1) * 512],
                    )
            if m != N_SB - 1:
                nc.sync.dma_start(e.yav_d[r0:r0 + 128, :], oav[:])


def _emit_rep(e, rep):
    nc, tc = e.nc, e.tc
    _emit_warmup(e, rep)
    _emit_loads(e, rep)
    with tc.tile_pool(name="psum1", bufs=4, space="PSUM") as psum1:
        _emit_q_own(e, rep, psum1)
        _emit_kv(e, psum1)
    with (
        tc.tile_pool(name="av", bufs=4, space="PSUM") as av_pool,
        tc.tile_pool(name="pss", bufs=2, space="PSUM") as pss_pool,
        tc.tile_pool(name="den", bufs=2, space="PSUM") as den_pool,
    ):
        _emit_scores(e, rep, pss_pool)
        _emit_av(e, av_pool, den_pool)


def build_nc(reps=1, sim_no_cc=False):
    # sim_no_cc: replace the AllGather with local DMAs of the same size so
    # the (single-core, collective-free) TimelineSim can schedule the program.
    nc = bass.Bass("TRN2", target_bir_lowering=False, debug=False,
                   num_devices=N_CORES)

    e = SimpleNamespace(nc=nc, sim_no_cc=sim_no_cc)
    e.xT_d = nc.dram_tensor("xT", [D, S], BF16, kind="ExternalInput")
    e.xkT_d = nc.dram_tensor("xkT", [D, SKL], BF16, kind="ExternalInput")
    e.wqhT_d = nc.dram_tensor("wqhT", [D, D // 2], BF16, kind="ExternalInput")
    e.wkT_d = nc.dram_tensor("wkT", [D, D], BF16, kind="ExternalInput")
    e.wvT_d = nc.dram_tensor("wvT", [D, D], BF16, kind="ExternalInput")
    # additive causal mask for the last 128 local keys of a superblock,
    # [128 keys, 256 queries]; data depends only on h
    e.mask_d = nc.dram_tensor("maskT", [128, SB], F32, kind="ExternalInput")
    e.yav_d = nc.dram_tensor("yav", [S, D], BF16, kind="ExternalOutput")
    e.yden_d = nc.dram_tensor("yden", [S, 1], F32, kind="ExternalOutput")

    with tile.TileContext(nc) as tc:
        e.tc = tc
        with (
            tc.tile_pool(name="xT", bufs=ND * NQC) as xT_pool,
            tc.tile_pool(name="xkT", bufs=ND) as xkT_pool,
            tc.tile_pool(name="w", bufs=2 * ND) as w_pool,
            tc.tile_pool(name="wqh", bufs=ND) as wqh_pool,
            tc.tile_pool(name="KT", bufs=NO) as KT_pool,
            tc.tile_pool(name="V", bufs=NSKL) as V_pool,
            tc.tile_pool(name="QT", bufs=NO) as QT_pool,
            tc.tile_pool(name="qe", bufs=4) as qe_pool,
            tc.tile_pool(name="mask", bufs=1) as mask_pool,
            tc.tile_pool(name="ones", bufs=1) as ones_pool,
            tc.tile_pool(name="es", bufs=3) as es_pool,
            tc.tile_pool(name="oav", bufs=6) as oav_pool,
            tc.tile_pool(name="oden", bufs=4) as oden_pool,
            tc.tile_pool(name="warm", bufs=2) as warm_pool,
            tc.tile_pool(name="qdram", bufs=2, space="DRAM") as qdram_pool,
        ):
            e.xT_pool, e.xkT_pool, e.w_pool, e.wqh_pool = (
                xT_pool, xkT_pool, w_pool, wqh_pool)
            e.KT_pool, e.V_pool, e.QT_pool, e.qe_pool = (
                KT_pool, V_pool, QT_pool, qe_pool)
            e.mask_pool, e.ones_pool, e.es_pool = mask_pool, ones_pool, es_pool
            e.oav_pool, e.oden_pool, e.warm_pool = oav_pool, oden_pool, warm_pool
            e.qdram_pool = qdram_pool
            for rep in range(reps):
                _emit_rep(e, rep)

    _legalize_waits(nc)
    return nc


_NC_CACHE = None


def _get_nc():
    global _NC_CACHE
    if _NC_CACHE is None:
        _NC_CACHE = build_nc()
    return _NC_CACHE


def _prep_core_inputs(x, wqT, wkT, wvT, b, h):
    xb = np.ascontiguousarray(x[b])                       # [S, D] fp32
    xT = np.ascontiguousarray(xb.T).astype(ml_dtypes.bfloat16)
    # keys interleave at 128-row granularity: core h owns global 128-blocks
    # {h, h+2, ..., h+14}; superblock m's local extent is exactly 128*(m+1)
    # for both cores, and only the last local key tile needs masking.
    blocks = [h + 2 * i for i in range(8)]
    xk = np.concatenate([xb[128 * t:128 * (t + 1)] for t in blocks], axis=0)
    xkT = np.ascontiguousarray(xk.T).astype(ml_dtypes.bfloat16)
    # mask for the last local key tile (global block 2m+h vs queries of
    # superblock m): keep iff 128*h + r_k <= r_q
    kk = np.arange(128)[:, None]
    qq = np.arange(SB)[None, :]
    maskT = np.where(128 * h + kk <= qq, 0.0, MASK_NEG).astype(np.float32)
    # Q-projection dedup: core h computes only o-dims [512h, 512h+512)
    wqhT = np.ascontiguousarray(wqT[:, 512 * h:512 * (h + 1)])
    return {
        "xT": xT, "xkT": xkT,
        "wqhT": wqhT, "wkT": wkT, "wvT": wvT,
        "maskT": maskT,
    }


def kernel(x, W_q, W_k, W_v):
    x = np.asarray(x, dtype=np.float32)
    wqT = np.ascontiguousarray(np.asarray(W_q, np.float32).T).astype(ml_dtypes.bfloat16)
    wkT = np.ascontiguousarray(np.asarray(W_k, np.float32).T).astype(ml_dtypes.bfloat16)
    wvT = np.ascontiguousarray(np.asarray(W_v, np.float32).T).astype(ml_dtypes.bfloat16)

    in_maps = []
    for c in range(N_CORES):
        b, h = divmod(c, 2)
        in_maps.append(_prep_core_inputs(x, wqT, wkT, wvT, b, h))

    nc = _get_nc()
    res = run_bass_kernel_spmd(nc, in_maps, list(range(N_CORES)))

    out = np.empty((B, S, D), dtype=np.float32)
    for b in range(B):
        av0 = np.asarray(res.results[2 * b]["yav"], dtype=np.float32)
        av1 = np.asarray(res.results[2 * b + 1]["yav"], dtype=np.float32)
        den = res.results[2 * b]["yden"] + res.results[2 * b + 1]["yden"]
        out[b] = (av0 + av1) / den
    return out


# revision 8
# speedup vs baseline: 1.5218x; 1.2730x over previous
"""Causal single-head attention on 8 trn2 NeuronCores.

Problem: x[4, 2048, 1024] fp32, W_q/W_k/W_v [1024, 1024] fp32 (torch Linear
layout, y = x @ W.T). Causal softmax attention, d_out = 1024.

Sharding: data-parallel over batch (4) x 2-way KEY split per batch.
Core c = 2*b + h handles batch b and the eight 128-row KEY blocks
{h, h+2, ..., h+14} (global 128-row block indices). Each core needs Q for
ALL 2048 queries but K/V only for its 1024 keys, runs flash-style
attention of all queries against its keys (exp without max subtraction —
scores are bounded — so partial sums merge exactly), and outputs the
unnormalized partial AV (bf16) and partial denominator (fp32). The host
merges: out = (AV_0 + AV_1) / (den_0 + den_1). Interleaving key blocks at
128-row granularity makes the local causal extent for query superblock m
exactly 128*(m+1) on BOTH cores of a pair — one SPMD program, zero static
extent waste at tile granularity, and only the last local key tile of each
superblock needs a causal mask, a single m-independent [128, 256] additive
tile whose data depends only on h (supplied as input).

Q-projection dedup: instead of both cores of a pair computing the full
Q^T (8 o-tiles), each core computes only 4 o-tiles — which ones is data:
core h receives wqhT = W_q^T[:, 512h:512(h+1)] — writes them to a DRAM
bounce buffer, and an in-pair AllGather assembles the full Q^T[1024, 2048]
(rank order puts o-dims 0..511 from core 2b first, so the gathered block
order is the natural o order on both cores). The gather + reload overlap
the V and K projections. This removes 65536 of ~410k PE cycles per core.

Per-core device program (matmul operands bf16, fp32 PSUM accumulate):
  1. Warmup: dependency-free matmuls keep the PE clock un-gated while the
     first DMAs land. Q-own: Q^T_own[512, 2048] from x^T (loaded in 512-
     column chunks so the first psum group completes after ~2 MB of DMA),
     evacuated to DRAM, AllGather within the pair, full Q^T DMA'd back.
     Meanwhile: V[skl, o], K^T[o, skl] from gathered-key x^T.
  2. Attention, key-stationary scores pass: per local key tile t, scores
     S^T[128, w] against all query superblocks m >= t in 512-wide matmul
     chunks, diagonal mask add, exp (ACT, scale 1/32) into resident bf16
     es tiles. Query-stationary AV pass: per superblock m, accumulate AV
     and the softmax denominator (PE ones-matmul) in PSUM over tiles
     0..m, evacuate av as bf16 + den as fp32.
"""

import copy
from types import SimpleNamespace

import numpy as np
import ml_dtypes

import concourse.bass as bass
import concourse.mybir as mybir
import concourse.tile as tile
from concourse.bass_utils import run_bass_kernel_spmd

BF16 = mybir.dt.bfloat16
F32 = mybir.dt.float32

B, S, D = 4, 2048, 1024
N_CORES = 8
SB = 256            # query superblock rows / key gather block
N_SB = S // SB      # 8 query superblocks per core
SKL = S // 2        # local keys per core (1024)
MASK_NEG = -1.0e5

ND = D // 128       # 8 d-tiles
NO = D // 128       # 8 o-tiles
NSKL = SKL // 128   # 8 local key tiles
NQC = S // 512      # 4 query column chunks


def _legalize_waits(nc):
    """Split multi-wait instructions into single-wait NOP chains.

    The walrus here accepts at most one sync-wait command per instruction,
    while TileContext emits several `on_wait` entries on one instruction.
    Hoist all but the last wait onto same-engine NOPs placed immediately
    before the instruction; the engine sequencer stalls on each in order.
    """
    uid = 0
    for fn in nc.m.functions:
        for bb in fn.blocks:
            out = []
            for inst in bb.instructions:
                si = inst.sync_info
                waits = list(si.on_wait) if si and si.on_wait else []
                if len(waits) > 1:
                    for w in waits[:-1]:
                        nop = mybir.InstNoOp(name=f"waitsplit_{uid}", ins=[], outs=[])
                        uid += 1
                        nop.engine = inst.engine
                        si2 = copy.deepcopy(si)
                        si2.on_wait = [w]
                        si2.on_update = []
                        nop.sync_info = si2
                        out.append(nop)
                    si.on_wait = waits[-1:]
                    inst.sync_info = si
                out.append(inst)
            bb.instructions = out


N_WARM = 6


def _emit_warmup(e, rep):
    # dependency-free matmuls keep PE busy during the initial DMA wait so
    # phase 1 starts at full clock; the zero source is written once (its
    # content is never read downstream)
    nc, tc = e.nc, e.tc
    if rep == 0:
        e.wsrc_t = e.warm_pool.tile([128, 512], BF16, tag="wsrc", name="wsrc")
        nc.gpsimd.memset(e.wsrc_t[:], 0.0)
    wsrc = e.wsrc_t
    with tc.tile_pool(name="wps", bufs=1, space="PSUM") as wps_pool:
        wps = wps_pool.tile([128, 512], F32, tag="wps", name=f"wps{rep}")
        for i in range(N_WARM):
            nc.tensor.matmul(
                wps[:], wsrc[:, 0:128], wsrc[:], start=(i == 0),
                stop=(i == N_WARM - 1),
            )


def _emit_loads(e, rep):
    # loads, in consumption order:
    # Q-own <- (wqh, xT chunks); V <- (wv, xk); K <- wk
    nc = e.nc
    e.wqh_t = []
    for i in range(ND):
        t = e.wqh_pool.tile([128, D // 2], BF16, tag="wqh", name=f"wqh{i}")
        nc.sync.dma_start(t[:], e.wqhT_d[i * 128:(i + 1) * 128, :])
        e.wqh_t.append(t)
    # xT in 512-column chunks, sqc-major so Q-own streams tile-by-tile
    e.xT_t = [[None] * ND for _ in range(NQC)]
    for sqc in range(NQC):
        for i in range(ND):
            t = e.xT_pool.tile([128, 512], BF16, tag="xT", name=f"xq{sqc}_{i}")
            nc.sync.dma_start(
                t[:], e.xT_d[i * 128:(i + 1) * 128, sqc * 512:(sqc + 1) * 512]
            )
            e.xT_t[sqc][i] = t
    e.wv_t = []
    for i in range(ND):
        t = e.w_pool.tile([128, D], BF16, tag="w", name=f"wv{i}")
        nc.sync.dma_start(t[:], e.wvT_d[i * 128:(i + 1) * 128, :])
        e.wv_t.append(t)
    e.xk_t = []
    for i in range(ND):
        t = e.xkT_pool.tile([128, SKL], BF16, tag="xkT", name=f"xk{i}")
        nc.sync.dma_start(t[:], e.xkT_d[i * 128:(i + 1) * 128, :])
        e.xk_t.append(t)
    e.wk_t = []
    for i in range(ND):
        t = e.w_pool.tile([128, D], BF16, tag="w", name=f"wk{i}")
        nc.sync.dma_start(t[:], e.wkT_d[i * 128:(i + 1) * 128, :])
        e.wk_t.append(t)
    if rep == 0:
        e.mask_t = e.mask_pool.tile([128, SB], F32, tag="mask", name="mask0")
        nc.sync.dma_start(e.mask_t[:], e.mask_d[0:128, :])
        e.ones_t = e.ones_pool.tile([128, 1], BF16)
        nc.gpsimd.memset(e.ones_t[:], 1.0)


def _emit_q_own(e, rep, psum1):
    # Q-own^T[o_own, sq]: 4 o-tiles of own half, all queries -> DRAM bounce
    nc = e.nc
    qin_d = e.qdram_pool.tile([D // 2, S], BF16, tag=f"qin{rep}", bufs=1,
                              name=f"qin{rep}")
    qout_d = e.qdram_pool.tile([D, S], BF16, tag=f"qout{rep}", bufs=1,
                               name=f"qout{rep}")
    for sqc in range(NQC):
        for ot in range(NO // 2):
            ps = psum1.tile([128, 512], F32, tag="ps1", name=f"psq{sqc}_{ot}")
            for d in range(ND):
                nc.tensor.matmul(
                    ps[:],
                    e.wqh_t[d][:, ot * 128:(ot + 1) * 128],
                    e.xT_t[sqc][d][:],
                    start=(d == 0),
                    stop=(d == ND - 1),
                )
            qe = e.qe_pool.tile([128, 512], BF16, tag="qe", name=f"qe{sqc}_{ot}")
            nc.vector.tensor_copy(qe[:], ps[:])
            nc.sync.dma_start(
                qin_d[ot * 128:(ot + 1) * 128, sqc * 512:(sqc + 1) * 512], qe[:]
            )
    # in-pair allgather: rank 2b's half (o 0..511) lands first, so gathered
    # block j == o-tile j on both cores
    if e.sim_no_cc:
        nc.sync.dma_start(qout_d[0:D // 2, :], qin_d[:])
        nc.sync.dma_start(qout_d[D // 2:D, :], qin_d[:])
    else:
        nc.gpsimd.collective_compute(
            "AllGather",
            mybir.AluOpType.bypass,
            replica_groups=[[0, 1], [2, 3], [4, 5], [6, 7]],
            ins=[qin_d.opt()],
            outs=[qout_d.opt()],
        )
    e.QT_t = [
        e.QT_pool.tile([128, S], BF16, tag="QT", name=f"QT{ot}")
        for ot in range(NO)
    ]
    for ot in range(NO):
        nc.sync.dma_start(e.QT_t[ot][:], qout_d[ot * 128:(ot + 1) * 128, :])


def _emit_kv(e, psum1):
    nc = e.nc
    # V[skl, o] = sum_d xkT[d, skl] * wvT[d, o]
    e.V_t = [e.V_pool.tile([128, D], BF16, tag="V", name=f"V{st}")
             for st in range(NSKL)]
    for st in range(NSKL):
        for oc in range(D // 512):
            ps = psum1.tile([128, 512], F32, tag="ps1", name=f"psv{st}_{oc}")
            for d in range(ND):
                nc.tensor.matmul(
                    ps[:],
                    e.xk_t[d][:, st * 128:(st + 1) * 128],
                    e.wv_t[d][:, oc * 512:(oc + 1) * 512],
                    start=(d == 0),
                    stop=(d == ND - 1),
                )
            nc.vector.tensor_copy(e.V_t[st][:, oc * 512:(oc + 1) * 512], ps[:])
    # K^T[o, skl] = sum_d wkT[d, o] * xkT[d, skl]
    e.KT_t = [e.KT_pool.tile([128, SKL], BF16, tag="KT", name=f"KT{ot}")
              for ot in range(NO)]
    for skc in range(SKL // 512):
        for ot in range(NO):
            ps = psum1.tile([128, 512], F32, tag="ps1", name=f"psk{skc}_{ot}")
            for d in range(ND):
                nc.tensor.matmul(
                    ps[:],
                    e.wk_t[d][:, ot * 128:(ot + 1) * 128],
                    e.xk_t[d][:, skc * 512:(skc + 1) * 512],
                    start=(d == 0),
                    stop=(d == ND - 1),
                )
            nc.vector.tensor_copy(e.KT_t[ot][:, skc * 512:(skc + 1) * 512], ps[:])


def _emit_scores(e, rep, pss_pool):
    # key-stationary scores pass: per local key tile t, scores against all
    # query superblocks m >= t in 512-wide chunks, mask + exp into resident
    # bf16 es tiles
    nc = e.nc
    e.es_t = []
    for t in range(NSKL):
        w = S - SB * t          # queries [SB*t, S)
        es = e.es_pool.tile([128, w], BF16, tag=f"es{t}", bufs=1,
                            name=f"es{rep}_{t}")
        e.es_t.append(es)
        for c0 in range(0, w, 512):
            cw = min(512, w - c0)
            ps_s = pss_pool.tile([128, 512], F32, tag="pss", name=f"pss{t}_{c0}")
            for od in range(NO):
                nc.tensor.matmul(
                    ps_s[:, 0:cw],
                    e.KT_t[od][:, t * 128:(t + 1) * 128],
                    e.QT_t[od][:, SB * t + c0:SB * t + c0 + cw],
                    start=(od == 0),
                    stop=(od == NO - 1),
                )
            if c0 == 0:
                # first 256 queries belong to superblock m=t: causal
                # boundary mask for the diagonal tile
                nc.vector.tensor_add(ps_s[:, 0:SB], ps_s[:, 0:SB], e.mask_t[:])
            nc.scalar.activation(
                es[:, c0:c0 + cw], ps_s[:, 0:cw],
                mybir.ActivationFunctionType.Exp,
                scale=1.0 / 32.0,
            )


def _emit_av(e, av_pool, den_pool):
    # query-stationary AV pass: per superblock m, accumulate AV and the
    # softmax denominator (PE ones-matmul) in PSUM over tiles 0..m
    nc = e.nc
    for m in range(N_SB):
        n = m + 1              # local sk-tiles this superblock
        avs = [av_pool.tile([128, 512], F32, tag="av", name=f"av{m}_{i}")
               for i in range(4)]
        dens = [den_pool.tile([128, 1], F32, tag="den", name=f"den{m}_{i}")
                for i in range(2)]
        # tile-major: finish each av tile's accumulation before starting the
        # next, so its evacuation overlaps the remaining AV matmuls instead
        # of serializing after the superblock's last matmul
        for sqh in range(2):
            for t in range(n):
                q0 = SB * (m - t)
                nc.tensor.matmul(
                    dens[sqh][:],
                    e.es_t[t][:, q0 + sqh * 128:q0 + (sqh + 1) * 128],
                    e.ones_t[:],
                    start=(t == 0),
                    stop=(t == n - 1),
                )
        for sqh in range(2):
            for oh in range(2):
                for t in range(n):
                    q0 = SB * (m - t)
                    nc.tensor.matmul(
                        avs[sqh * 2 + oh][:],
                        e.es_t[t][:, q0 + sqh * 128:q0 + (sqh + 1) * 128],
                        e.V_t[t][:, oh * 512:(oh + 1) * 512],
                        start=(t == 0),
                        stop=(t == n - 1),
                    )
        for sqh in range(2):
            r0 = m * SB + sqh * 128
            oden = e.oden_pool.tile([128, 1], F32, tag="oden",
                                    name=f"oden{m}_{sqh}")
            nc.vector.tensor_copy(oden[:], dens[sqh][:])
            nc.sync.dma_start(e.yden_d[r0:r0 + 128, :], oden[:])
            oav = e.oav_pool.tile([128, D], BF16, tag="oav", name=f"oav{m}_{sqh}")
            for oh in range(2):
                nc.vector.tensor_copy(
                    oav[:, oh * 512:(oh + 1) * 512], avs[sqh * 2 + oh][:]
                )
                if m == N_SB - 1:
                    # kernel tail: ship each half as soon as its copy lands
                    nc.sync.dma_start(
                        e.yav_d[r0:r0 + 128, oh * 512:(oh + 1) * 512],
                        oav[:, oh * 512:(oh + 1) * 512],
                    )
            if m != N_SB - 1:
                nc.sync.dma_start(e.yav_d[r0:r0 + 128, :], oav[:])


def _emit_rep(e, rep):
    nc, tc = e.nc, e.tc
    _emit_warmup(e, rep)
    _emit_loads(e, rep)
    with tc.tile_pool(name="psum1", bufs=4, space="PSUM") as psum1:
        _emit_q_own(e, rep, psum1)
        _emit_kv(e, psum1)
    with (
        tc.tile_pool(name="av", bufs=4, space="PSUM") as av_pool,
        tc.tile_pool(name="pss", bufs=2, space="PSUM") as pss_pool,
        tc.tile_pool(name="den", bufs=2, space="PSUM") as den_pool,
    ):
        _emit_scores(e, rep, pss_pool)
        _emit_av(e, av_pool, den_pool)


def build_nc(reps=1, sim_no_cc=False):
    # sim_no_cc: replace the AllGather with local DMAs of the same size so
    # the (single-core, collective-free) TimelineSim can schedule the program.
    nc = bass.Bass("TRN2", target_bir_lowering=False, debug=False,
                   num_devices=N_CORES)

    e = SimpleNamespace(nc=nc, sim_no_cc=sim_no_cc)
    e.xT_d = nc.dram_tensor("xT", [D, S], BF16, kind="ExternalInput")
    e.xkT_d = nc.dram_tensor("xkT", [D, SKL], BF16, kind="ExternalInput")
    e.wqhT_d = nc.dram_tensor("wqhT", [D, D // 2], BF16, kind="ExternalInput")
    e.wkT_d = nc.dram_tensor("wkT", [D, D], BF16, kind="ExternalInput")
    e.wvT_d = nc.dram_tensor("wvT", [D, D], BF16, kind="ExternalInput")
    # additive causal mask for the last 128 local keys of a superblock,
    # [128 keys, 256 queries]; data depends only on h
    e.mask_d = nc.dram_tensor("maskT", [128, SB], F32, kind="ExternalInput")
    e.yav_d = nc.dram_tensor("yav", [S, D], BF16, kind="ExternalOutput")
    e.yden_d = nc.dram_tensor("yden", [S, 1], F32, kind="ExternalOutput")

    with tile.TileContext(nc) as tc:
        e.tc = tc
        with (
            tc.tile_pool(name="xT", bufs=ND * NQC) as xT_pool,
            tc.tile_pool(name="xkT", bufs=ND) as xkT_pool,
            tc.tile_pool(name="w", bufs=2 * ND) as w_pool,
            tc.tile_pool(name="wqh", bufs=ND) as wqh_pool,
            tc.tile_pool(name="KT", bufs=NO) as KT_pool,
            tc.tile_pool(name="V", bufs=NSKL) as V_pool,
            tc.tile_pool(name="QT", bufs=NO) as QT_pool,
            tc.tile_pool(name="qe", bufs=4) as qe_pool,
            tc.tile_pool(name="mask", bufs=1) as mask_pool,
            tc.tile_pool(name="ones", bufs=1) as ones_pool,
            tc.tile_pool(name="es", bufs=3) as es_pool,
            tc.tile_pool(name="oav", bufs=6) as oav_pool,
            tc.tile_pool(name="oden", bufs=4) as oden_pool,
            tc.tile_pool(name="warm", bufs=2) as warm_pool,
            tc.tile_pool(name="qdram", bufs=2, space="DRAM") as qdram_pool,
        ):
            e.xT_pool, e.xkT_pool, e.w_pool, e.wqh_pool = (
                xT_pool, xkT_pool, w_pool, wqh_pool)
            e.KT_pool, e.V_pool, e.QT_pool, e.qe_pool = (
                KT_pool, V_pool, QT_pool, qe_pool)
            e.mask_pool, e.ones_pool, e.es_pool = mask_pool, ones_pool, es_pool
            e.oav_pool, e.oden_pool, e.warm_pool = oav_pool, oden_pool, warm_pool
            e.qdram_pool = qdram_pool
            for rep in range(reps):
                _emit_rep(e, rep)

    _legalize_waits(nc)
    return nc


_NC_CACHE = None


def _get_nc():
    global _NC_CACHE
    if _NC_CACHE is None:
        _NC_CACHE = build_nc()
    return _NC_CACHE


def _prep_core_inputs(x, wqT, wkT, wvT, b, h):
    xb = np.ascontiguousarray(x[b])                       # [S, D] fp32
    xT = np.ascontiguousarray(xb.T).astype(ml_dtypes.bfloat16)
    # keys interleave at 128-row granularity: core h owns global 128-blocks
    # {h, h+2, ..., h+14}; superblock m's local extent is exactly 128*(m+1)
    # for both cores, and only the last local key tile needs masking.
    blocks = [h + 2 * i for i in range(8)]
    xk = np.concatenate([xb[128 * t:128 * (t + 1)] for t in blocks], axis=0)
    xkT = np.ascontiguousarray(xk.T).astype(ml_dtypes.bfloat16)
    # mask for the last local key tile (global block 2m+h vs queries of
    # superblock m): keep iff 128*h + r_k <= r_q
    kk = np.arange(128)[:, None]
    qq = np.arange(SB)[None, :]
    maskT = np.where(128 * h + kk <= qq, 0.0, MASK_NEG).astype(np.float32)
    # Q-projection dedup: core h computes only o-dims [512h, 512h+512)
    wqhT = np.ascontiguousarray(wqT[:, 512 * h:512 * (h + 1)])
    return {
        "xT": xT, "xkT": xkT,
        "wqhT": wqhT, "wkT": wkT, "wvT": wvT,
        "maskT": maskT,
    }


def kernel(x, W_q, W_k, W_v):
    x = np.asarray(x, dtype=np.float32)
    wqT = np.ascontiguousarray(np.asarray(W_q, np.float32).T).astype(ml_dtypes.bfloat16)
    wkT = np.ascontiguousarray(np.asarray(W_k, np.float32).T).astype(ml_dtypes.bfloat16)
    wvT = np.ascontiguousarray(np.asarray(W_v, np.float32).T).astype(ml_dtypes.bfloat16)

    in_maps = []
    for c in range(N_CORES):
        b, h = divmod(c, 2)
        in_maps.append(_prep_core_inputs(x, wqT, wkT, wvT, b, h))

    nc = _get_nc()
    res = run_bass_kernel_spmd(nc, in_maps, list(range(N_CORES)))

    out = np.empty((B, S, D), dtype=np.float32)
    for b in range(B):
        av0 = np.asarray(res.results[2 * b]["yav"], dtype=np.float32)
        av1 = np.asarray(res.results[2 * b + 1]["yav"], dtype=np.float32)
        den = res.results[2 * b]["yden"] + res.results[2 * b + 1]["yden"]
        out[b] = (av0 + av1) / den
    return out
